# revision 1
# baseline (speedup 1.0000x reference)
"""Trainium2 Bass kernel for the CGIM sparse-attention block.

Per-sample math (reference):
  Qf = Wq @ [F1;F2] + bq            (1x1 conv, transposed-layout on device)
  Qs = softmax_d(Qf per head)
  per branch i: K = Wk_i @ F_i (+bk_i cancels), V = Wv_i @ F_i + bv_i
                Ks = softmax_hw(K);  Att = Ks @ Qs;  Xw = Att @ V
  fused = concat(mu*X1 + F1, mu*X2 + F2)
  y = relu(BN(conv3x3(fused, Wc)))

Sharding: data-parallel over batch (B=8) across the 8 NeuronCores; weights
replicated. Each core computes one sample end to end; no collectives.

Key device-side algebra:
 - K softmax bias cancels (constant along softmax axis) -> dropped.
 - K softmax denominator S_d is postponed all the way to the Xw epilogue,
   where d is the partition axis (fused scale mu/S_d).
 - Q/K computed directly in transposed [hw, c] layout (stationary = F tile),
   so Q's per-head softmax is a free-dim segmented reduce and the Att
   contraction over hw needs no transposes at all.
 - AttT computed directly as lhsT for the Xw matmul; only the 32x32
   diagonal (per-head) blocks are copied into a zeroed block-diag tile.
 - conv3x3 = 36 accumulated shifted 1x1 matmuls over a zero-padded
   [66 x 72] image layout; BN+ReLU folded into the PSUM->SBUF ACT.
All matmul operands bf16, fp32 PSUM accumulation.
"""

import numpy as np
import ml_dtypes

import concourse.bass as bass
import concourse.mybir as mybir
import concourse.tile as tile
from concourse import bacc
from concourse.bass_utils import run_bass_kernel_spmd

BF16 = mybir.dt.bfloat16
F32 = mybir.dt.float32
AF = mybir.ActivationFunctionType
ALU = mybir.AluOpType
AX = mybir.AxisListType

B, C, H, W = 8, 256, 64, 64
HW = H * W                  # 4096
NH, D = 8, 32               # heads, per-head dim
NT = HW // 128              # 32 hw-tiles of 128
PH, PW = H + 2, 72          # padded conv image (66 rows x 72 cols, >=66 used)
N_CORES = 8
BN_EPS = 1e-5

_bf = ml_dtypes.bfloat16


def _build_program() -> bass.Bass:
    nc = bacc.Bacc("TRN2", target_bir_lowering=False)

    # ---- DRAM I/O (per core) ----
    f1_d = nc.dram_tensor("f1", [C, HW], BF16, kind="ExternalInput").ap()
    f2_d = nc.dram_tensor("f2", [C, HW], BF16, kind="ExternalInput").ap()
    wq_d = nc.dram_tensor("wq", [128, 4, 256], BF16, kind="ExternalInput").ap()
    wk_d = nc.dram_tensor("wk", [128, 2, 2, 256], BF16, kind="ExternalInput").ap()
    wv_d = nc.dram_tensor("wv", [128, 2, 2, 256], BF16, kind="ExternalInput").ap()
    wc_d = nc.dram_tensor("wc", [128, 4, 18, 128], BF16, kind="ExternalInput").ap()
    bq_d = nc.dram_tensor("bq", [1, 256], BF16, kind="ExternalInput").ap()
    bv_d = nc.dram_tensor("bv", [128, 2, 2], F32, kind="ExternalInput").ap()
    bns_d = nc.dram_tensor("bns", [128, 2], F32, kind="ExternalInput").ap()
    bnb_d = nc.dram_tensor("bnb", [128, 2], F32, kind="ExternalInput").ap()
    muv_d = nc.dram_tensor("muv", [128, 1], F32, kind="ExternalInput").ap()
    y_d = nc.dram_tensor("y", [C, HW], F32, kind="ExternalOutput").ap()

    with tile.TileContext(nc) as tc:
        with tc.tile_pool(name="per", bufs=1) as per, \
             tc.tile_pool(name="sml", bufs=4) as sml:

            # ---- persistent SBUF tiles ----
            wq = per.tile([128, 4, 256], BF16)
            wk = per.tile([128, 2, 2, 256], BF16)
            wv = per.tile([128, 2, 2, 256], BF16)
            wc = per.tile([128, 4, 18, 128], BF16)
            bq = per.tile([1, 256], BF16)
            bv = per.tile([128, 2, 2], F32)
            bns = per.tile([128, 2], F32)
            bnb = per.tile([128, 2], F32)
            muv = per.tile([128, 1], F32)

            ones_row = per.tile([1, 128], BF16)
            nc.vector.memset(ones_row, 1.0)
            ones_col = per.tile([128, 1], BF16)
            nc.gpsimd.memset(ones_col, 1.0)

            fbf = per.tile([128, 4, HW], BF16)        # [F1;F2] as 4 ci-tiles
            # qk[:, n, 0:256]=exp(QfT) (normalized in place),
            # [:, n, 256:512]=exp(K1fT), [:, n, 512:768]=exp(K2fT)
            qk = per.tile([128, NT, 768], BF16)
            vsb1 = per.tile([128, 2, HW], BF16)       # V1, 2 m-groups
            vsb2 = per.tile([128, 2, HW], BF16)
            fp = [per.tile([128, PH, PW], BF16, tag=f"fp{j}", name=f"fp{j}")
                  for j in range(4)]

            # DMA *issue* on one sequencer costs ~1us per dma_start, so the
            # preamble is issue-limited: spread issue across the idle
            # vector/scalar sequencers for weights and sync+gpsimd for F.
            for ci in range(4):
                nc.scalar.dma_start(wq[:, ci, :], wq_d[:, ci, :])
            nc.scalar.dma_start(bq, bq_d)

            f_src = [f1_d, f1_d, f2_d, f2_d]
            bounds = [0, 512, 1536, 2560, 3584, 4096]
            k = 0
            for ch in range(len(bounds) - 1):
                lo, hi = bounds[ch], bounds[ch + 1]
                for ci in range(4):
                    half = (ci % 2) * 128
                    eng = nc.sync if k % 2 == 0 else nc.gpsimd
                    eng.dma_start(fbf[:, ci, lo:hi],
                                  f_src[ci][half:half + 128, lo:hi])
                    k += 1
                if ch == 0:
                    nc.sync.dma_start(wk[:, 0, 0, :], wk_d[:, 0, 0, :])
                    nc.gpsimd.dma_start(wk[:, 0, 1, :], wk_d[:, 0, 1, :])
                    nc.sync.dma_start(wk[:, 1, 0, :], wk_d[:, 1, 0, :])
                    nc.gpsimd.dma_start(wk[:, 1, 1, :], wk_d[:, 1, 1, :])

            for br in range(2):
                for ci in range(2):
                    nc.sync.dma_start(wv[:, br, ci, :], wv_d[:, br, ci, :])
            nc.sync.dma_start(bv, bv_d)
            nc.sync.dma_start(muv, muv_d)
            for j in range(4):
                nc.gpsimd.memset(fp[j], 0.0)

            # ================= Phase 1: transposed Q/K1/K2 + softmax pieces
            # One [128,768] PSUM tile spans 2 banks: Q(0:256)+K1(256:512) in
            # bank A, K2(512:768) in bank B. One start/stop per bank (first/
            # last matmul touching it); other first-writes rely on per-element
            # has_written. This lets ONE ACT do all three exps (the 3-op
            # version saturates ScalarE and paces phase 1).
            pv_ctx = tc.tile_pool(name="pv", bufs=3, space="PSUM")
            pv = pv_ctx.__enter__()
            with tc.tile_pool(name="pq", bufs=2, space="PSUM") as pq, \
                 tc.tile_pool(name="pss", bufs=1, space="PSUM") as pss:

                ps_s = pss.tile([1, 512], F32, tag="s")

                LAG = 2
                def emit_ssum(n):
                    nc.tensor.matmul(ps_s, ones_col, qk[:, n, 256:768],
                                     start=(n == 0), stop=(n == NT - 1))

                gk = dict(skip_group_check=True)
                for n in range(NT):
                    pqk = pq.tile([128, 768], F32, tag="qk")
                    psq, psk1, psk2 = pqk[:, 0:256], pqk[:, 256:512], pqk[:, 512:768]
                    for ci in range(4):
                        lhsT = fbf[:, ci, n * 128:(n + 1) * 128]
                        nc.tensor.matmul(psq, lhsT, wq[:, ci, :],
                                         start=(ci == 0), stop=False, **gk)
                        if ci < 2:
                            nc.tensor.matmul(psk1, lhsT, wk[:, 0, ci, :],
                                             start=False, stop=False, **gk)
                        else:
                            nc.tensor.matmul(psk2, lhsT, wk[:, 1, ci - 2, :],
                                             start=(ci == 2), stop=(ci == 3), **gk)
                    nc.tensor.matmul(psq, ones_row, bq, start=False, stop=True, **gk)

                    act = nc.scalar.activation(qk[:, n, :], pqk, AF.Exp)
                    if n == 8:
                        wc_anchor = act

                    # per-head softmax denominator + normalize (in place)
                    q3 = qk[:, n, 0:256].rearrange("p (h e) -> p h e", h=NH)
                    rq = sml.tile([128, NH], F32, tag="rq")
                    nc.vector.tensor_reduce(rq, q3, axis=AX.X, op=ALU.add)
                    rr = sml.tile([128, NH], F32, tag="rr")
                    nc.vector.reciprocal(rr, rq)
                    nc.vector.tensor_mul(q3, q3, rr.to_broadcast([128, NH, D]))

                    if n >= LAG:
                        emit_ssum(n - LAG)
                for n in range(NT - LAG, NT):
                    emit_ssum(n)

                # 1/S row -> per-partition columns (tiny SBUF->SBUF DMAs)
                scale = {}
                rs = sml.tile([1, 512], F32, tag="rs")
                nc.vector.reciprocal(rs, ps_s)
                for br in range(2):
                    for m in range(2):
                        col = sml.tile([128, 1], F32, tag="scat")
                        nc.sync.dma_start(
                            col, rs[0:1, br * 256 + m * 128:br * 256 + (m + 1) * 128])
                        sc = sml.tile([128, 1], F32, tag="scale")
                        nc.vector.tensor_mul(sc, col, muv)   # mu / S_d
                        scale[(br, m)] = sc

            # conv weights: start loading mid-phase-1 (nosync dep keeps them
            # out of the preamble DMA window), done long before the conv.
            from concourse.tile import add_dep_helper
            for ci in range(4):
                d = nc.sync.dma_start(wc[:, ci, :, :], wc_d[:, ci, :, :])
                add_dep_helper(d.ins, wc_anchor.ins, sync=False,
                               reason="defer wc load past preamble")
            d = nc.sync.dma_start(bns, bns_d)
            add_dep_helper(d.ins, wc_anchor.ins, sync=False, reason="defer")
            d = nc.sync.dma_start(bnb, bnb_d)
            add_dep_helper(d.ins, wc_anchor.ins, sync=False, reason="defer")

            # ================= Phase 2: V convs + AttT + block-diag
            def emit_v(pv, br, vsb, fci0):
                for m in range(2):
                    for n8 in range(8):
                        psv = pv.tile([128, 512], F32, tag="v",
                                      name=f"psv{br}{m}{n8}")
                        for ci in range(2):
                            nc.tensor.matmul(
                                psv, wv[:, br, ci, m * 128:(m + 1) * 128],
                                fbf[:, fci0 + ci, n8 * 512:(n8 + 1) * 512],
                                start=(ci == 0), stop=(ci == 1))
                        nc.scalar.activation(
                            vsb[:, m, n8 * 512:(n8 + 1) * 512], psv,
                            AF.Identity, bias=bv[:, br, m:m + 1])

            emit_v(pv, 0, vsb1, 0)

            with tc.tile_pool(name="pa", bufs=4, space="PSUM") as pa:
                psa = {}
                for g in range(2):
                    for br in range(2):
                        p = pa.tile([128, 128], F32, tag="a", name=f"psa{br}{g}")
                        psa[(br, g)] = p
                for g in range(2):
                    for n in range(NT):
                        lhsT = qk[:, n, g * 128:(g + 1) * 128]
                        for br in range(2):
                            nc.tensor.matmul(
                                psa[(br, g)], lhsT,
                                qk[:, n, 256 + br * 256 + g * 128:
                                   256 + br * 256 + (g + 1) * 128],
                                start=(n == 0), stop=(n == NT - 1))

                emit_v(pv, 1, vsb2, 2)

                attbd = {}
                for (br, g), p in psa.items():
                    t = sml.tile([128, 128], BF16, tag="attbd")
                    nc.vector.memset(t, 0.0)
                    for hb in range(4):
                        hs = slice(hb * 32, (hb + 1) * 32)
                        nc.any.tensor_copy(t[hs, hs], p[hs, hs])
                    attbd[(br, g)] = t
            pv_ctx.__exit__(None, None, None)

            # ================= Phase 2b: Xw + fused epilogue
            with tc.tile_pool(name="px", bufs=4, space="PSUM") as px:
                for br, (vsb, fci0) in enumerate(((vsb1, 0), (vsb2, 2))):
                    for g in range(2):
                        for n8 in range(8):
                            pxt = px.tile([128, 512], F32, tag="x")
                            nc.tensor.matmul(
                                pxt, attbd[(br, g)],
                                vsb[:, g, n8 * 512:(n8 + 1) * 512],
                                start=True, stop=True)
                            # fused = (Xw_raw * mu/S_d) + F  -> padded layout
                            j = 2 * br + g
                            out = fp[j][:, 1 + n8 * 8:9 + n8 * 8, 1:65]
                            nc.vector.scalar_tensor_tensor(
                                out=out,
                                in0=pxt, scalar=scale[(br, g)],
                                in1=fbf[:, fci0 + g, n8 * 512:(n8 + 1) * 512],
                                op0=ALU.mult, op1=ALU.add)

            # ================= Phase 3: conv3x3 + BN + ReLU
            with tc.tile_pool(name="pc", bufs=8, space="PSUM") as pc:
                for m in range(2):
                    for hf in range(2):
                        pst = [pc.tile([128, 512], F32, tag="c",
                                       name=f"psc{m}{hf}{i}") for i in range(4)]
                        first, last = (0, 0, 0), (3, 2, 2)
                        for ci in range(4):
                            for dy in range(3):
                                for dx in range(3):
                                    lhsT = wc[:, ci, (dy * 3 + dx) * 2 + m, :]
                                    for i4 in range(4):
                                        n8 = hf * 4 + i4
                                        rhs = fp[ci][:, n8 * 8 + dy:n8 * 8 + dy + 8,
                                                     dx:dx + 64]
                                        nc.tensor.matmul(
                                            pst[i4], lhsT, rhs,
                                            start=((ci, dy, dx) == first),
                                            stop=((ci, dy, dx) == last))
                        for i4 in range(4):
                            n8 = hf * 4 + i4
                            ysb = sml.tile([128, 512], F32, tag="y")
                            nc.scalar.activation(ysb, pst[i4], AF.Relu,
                                                 bias=bnb[:, m:m + 1],
                                                 scale=bns[:, m:m + 1])
                            eng = nc.sync if n8 % 2 == 0 else nc.gpsimd
                            eng.dma_start(
                                y_d[m * 128:(m + 1) * 128,
                                    n8 * 512:(n8 + 1) * 512], ysb)
    nc.compile()
    return nc


_PROGRAM = None


def _get_program():
    global _PROGRAM
    if _PROGRAM is None:
        _PROGRAM = _build_program()
    return _PROGRAM


def kernel(F1, F2, Wq, bq, Wk1, bk1, Wv1, bv1, Wk2, bk2, Wv2, bv2,
           mu, Wc, gamma, beta, rmean, rvar):
    import os
    import sys
    if "antenv.axon_hooks" not in sys.modules:
        try:
            import antenv.axon_hooks  # noqa: F401
        except ImportError:
            # no profiling hook available: make sure a stray BASS_TRACE
            # can't route run_bass_kernel_spmd into the hook import
            os.environ["BASS_NEVER_TRACE"] = "1"
    f32 = np.float32
    F1 = np.asarray(F1, f32)
    F2 = np.asarray(F2, f32)

    def tile_T(w):   # [O, Cin] -> [128, Cin//128, O] (lhsT tiles)
        wt = np.ascontiguousarray(np.asarray(w, f32).T)      # [Cin, O]
        cin, o = wt.shape
        return wt.reshape(cin // 128, 128, o).transpose(1, 0, 2).astype(_bf)

    wq_h = np.ascontiguousarray(tile_T(Wq))                  # [128, 4, 256]
    wk_h = np.ascontiguousarray(
        np.stack([tile_T(Wk1), tile_T(Wk2)], axis=1))        # [128,2,2,256]
    wv_h = np.ascontiguousarray(
        np.stack([tile_T(Wv1), tile_T(Wv2)], axis=1))

    Wc = np.asarray(Wc, f32)                                 # [256, 512, 3, 3]
    # wc[p, ci, (dy*3+dx)*2+m, col] = Wc[m*128+col, ci*128+p, dy, dx]
    wc_h = Wc.reshape(2, 128, 4, 128, 3, 3)                  # m,col,ci,p,dy,dx
    wc_h = wc_h.transpose(3, 2, 4, 5, 0, 1)                  # p,ci,dy,dx,m,col
    wc_h = np.ascontiguousarray(
        wc_h.reshape(128, 4, 18, 128)).astype(_bf)

    bq_h = np.asarray(bq, f32).reshape(1, 256).astype(_bf)
    # bv_h[p, br, m] = bv_br[m*128 + p]
    bv_h = np.ascontiguousarray(
        np.stack([np.asarray(bv1, f32), np.asarray(bv2, f32)],
                 axis=0).reshape(2, 2, 128).transpose(2, 0, 1))
    inv = np.asarray(gamma, f32) / np.sqrt(np.asarray(rvar, f32) + BN_EPS)
    b2 = np.asarray(beta, f32) - np.asarray(rmean, f32) * inv
    bns_h = np.ascontiguousarray(inv.reshape(2, 128).T)      # [128, 2]
    bnb_h = np.ascontiguousarray(b2.reshape(2, 128).T)
    muv_h = np.full((128, 1), np.asarray(mu, f32).reshape(-1)[0], f32)

    shared = dict(wq=wq_h, wk=wk_h, wv=wv_h, wc=wc_h, bq=bq_h, bv=bv_h,
                  bns=bns_h, bnb=bnb_h, muv=muv_h)
    in_maps = [dict(f1=np.ascontiguousarray(F1[b].reshape(C, HW)).astype(_bf),
                    f2=np.ascontiguousarray(F2[b].reshape(C, HW)).astype(_bf),
                    **shared) for b in range(N_CORES)]

    nc = _get_program()
    res = run_bass_kernel_spmd(nc, in_maps, list(range(N_CORES)))
    kernel.last_results = res

    out = np.stack([res.results[b]["y"] for b in range(N_CORES)])
    return out.reshape(B, C, H, W)


kernel.last_results = None



# revision 6
# speedup vs baseline: 1.1232x; 1.1232x over previous
"""Trainium2 Bass kernel for the CGIM sparse-attention block.

Per-sample math (reference):
  Qf = Wq @ [F1;F2] + bq            (1x1 conv, transposed-layout on device)
  Qs = softmax_d(Qf per head)
  per branch i: K = Wk_i @ F_i (+bk_i cancels), V = Wv_i @ F_i + bv_i
                Ks = softmax_hw(K);  Att = Ks @ Qs;  Xw = Att @ V
  fused = concat(mu*X1 + F1, mu*X2 + F2)
  y = relu(BN(conv3x3(fused, Wc)))

Sharding: data-parallel over batch (B=8) across the 8 NeuronCores; weights
replicated. Each core computes one sample end to end; no collectives.

Key device-side algebra:
 - K softmax bias cancels (constant along softmax axis) -> dropped.
 - K softmax denominator S_d is postponed all the way to the Xw epilogue,
   where d is the partition axis (fused scale mu/S_d).
 - Q/K computed directly in transposed [hw, c] layout (stationary = F tile),
   so Q's per-head softmax is a free-dim segmented reduce and the Att
   contraction over hw needs no transposes at all.
 - AttT computed directly as lhsT for the Xw matmul; only the 32x32
   diagonal (per-head) blocks are copied into a zeroed block-diag tile.
 - conv3x3 = 36 accumulated shifted 1x1 matmuls over a zero-padded
   [66 x 72] image layout; BN+ReLU folded into the PSUM->SBUF ACT.
All matmul operands bf16, fp32 PSUM accumulation.

Scheduling notes (perf):
 - Q-bias and K-colsum matmuls use a full 128x128 ones matrix as lhsT:
   1-row/1-col stationaries force row_grp/col_grp masks and each mask
   switch costs ~+100ns on the next matmul.
 - Dummy warmup matmuls run during the DMA preamble so the PE HAM clock
   gate is already at 8/8 when phase 1 starts.
 - Xw chunks are emitted n8-major and the conv3x3 is emitted per-output-
   chunk (36-matmul accumulation groups) interleaved with them, so the
   PE never waits on the Vector/GpSimd epilogue and the conv tail
   (ACT+DMA) pipelines under the next group's matmuls.
 - The fused-residual epilogue alternates Vector / GpSimd so neither
   paces the PE.
"""

import numpy as np
import ml_dtypes

import concourse.bass as bass
import concourse.mybir as mybir
import concourse.tile as tile
from concourse import bacc
from concourse.bass_utils import run_bass_kernel_spmd

BF16 = mybir.dt.bfloat16
F32 = mybir.dt.float32
AF = mybir.ActivationFunctionType
ALU = mybir.AluOpType
AX = mybir.AxisListType

B, C, H, W = 8, 256, 64, 64
HW = H * W                  # 4096
NH, D = 8, 32               # heads, per-head dim
NT = HW // 128              # 32 hw-tiles of 128
PH, PW = H + 2, 72          # padded conv image (66 rows x 72 cols)
N_CORES = 8
BN_EPS = 1e-5

_bf = ml_dtypes.bfloat16


def _build_program() -> bass.Bass:
    nc = bacc.Bacc("TRN2", target_bir_lowering=False)

    # ---- DRAM I/O (per core) ----
    f1_d = nc.dram_tensor("f1", [C, HW], BF16, kind="ExternalInput").ap()
    f2_d = nc.dram_tensor("f2", [C, HW], BF16, kind="ExternalInput").ap()
    wq_d = nc.dram_tensor("wq", [128, 4, 256], BF16, kind="ExternalInput").ap()
    wk_d = nc.dram_tensor("wk", [128, 2, 2, 256], BF16, kind="ExternalInput").ap()
    wv_d = nc.dram_tensor("wv", [128, 2, 2, 256], BF16, kind="ExternalInput").ap()
    wc_d = nc.dram_tensor("wc", [128, 4, 18, 128], BF16, kind="ExternalInput").ap()
    bqr_d = nc.dram_tensor("bqr", [128, 256], BF16, kind="ExternalInput").ap()
    bv_d = nc.dram_tensor("bv", [128, 2, 2], F32, kind="ExternalInput").ap()
    bns_d = nc.dram_tensor("bns", [128, 2], F32, kind="ExternalInput").ap()
    bnb_d = nc.dram_tensor("bnb", [128, 2], F32, kind="ExternalInput").ap()
    muv_d = nc.dram_tensor("muv", [128, 1], F32, kind="ExternalInput").ap()
    y_d = nc.dram_tensor("y", [C, HW], F32, kind="ExternalOutput").ap()

    with tile.TileContext(nc) as tc:
        with tc.tile_pool(name="per", bufs=1) as per, \
             tc.tile_pool(name="sml", bufs=4) as sml:

            # ---- persistent SBUF tiles ----
            wq = per.tile([128, 4, 256], BF16)
            wk = per.tile([128, 2, 2, 256], BF16)
            wv = per.tile([128, 2, 2, 256], BF16)
            wc = per.tile([128, 4, 18, 128], BF16)
            bqr = per.tile([128, 256], BF16)
            bv = per.tile([128, 2, 2], F32)
            bns = per.tile([128, 2], F32)
            bnb = per.tile([128, 2], F32)
            muv = per.tile([128, 1], F32)

            ones = per.tile([128, 128], BF16)
            nc.vector.memset(ones, 1.0)

            fbf = per.tile([128, 4, HW], BF16)        # [F1;F2] as 4 ci-tiles
            # qk[:, n, 0:256]=exp(QfT) (normalized in place),
            # [:, n, 256:512]=exp(K1fT), [:, n, 512:768]=exp(K2fT)
            qk = per.tile([128, NT, 768], BF16)
            vsb1 = per.tile([128, 2, HW], BF16)       # V1, 2 m-groups
            vsb2 = per.tile([128, 2, HW], BF16)
            fp = [per.tile([128, PH, PW], BF16, tag=f"fp{j}", name=f"fp{j}")
                  for j in range(4)]
            attbd = [per.tile([128, 128], BF16, tag=f"abd{j}", name=f"abd{j}")
                     for j in range(4)]

            # PE warmup: dummy matmuls during the DMA preamble keep the PE
            # busy through the HAM window so phase 1 starts at 2.4 GHz.
            warm_ctx = tc.tile_pool(name="warm", bufs=1, space="PSUM")
            warm = warm_ctx.__enter__()
            wt = warm.tile([128, 64], F32)
            for _ in range(36):
                nc.tensor.matmul(wt, ones, ones[:, 0:64],
                                 start=True, stop=True)
            warm_ctx.__exit__(None, None, None)

            # ---- preamble DMA: first-needed tiles fan out across queues ----
            # DMA *issue* costs ~0.65us per dma_start per sequencer, so the
            # first phase-1 deps go one-per-queue, then bulk loads follow.
            f_src = [f1_d, f1_d, f2_d, f2_d]
            first_q = [nc.sync, nc.gpsimd, nc.scalar, nc.sync]
            for ci in range(4):
                half = (ci % 2) * 128
                first_q[ci].dma_start(fbf[:, ci, 0:128],
                                      f_src[ci][half:half + 128, 0:128])
            nc.scalar.dma_start(wq, wq_d)
            nc.scalar.dma_start(bqr, bqr_d)
            nc.sync.dma_start(wk[:, 0], wk_d[:, 0])
            nc.gpsimd.dma_start(wk[:, 1], wk_d[:, 1])

            bounds = [128, 1152, 2176, 3200, 4096]
            k = 0
            for ch in range(len(bounds) - 1):
                lo, hi = bounds[ch], bounds[ch + 1]
                for ci in range(4):
                    half = (ci % 2) * 128
                    eng = nc.sync if k % 2 == 0 else nc.gpsimd
                    eng.dma_start(fbf[:, ci, lo:hi],
                                  f_src[ci][half:half + 128, lo:hi])
                    k += 1

            for br in range(2):
                eng = nc.sync if br == 0 else nc.gpsimd
                eng.dma_start(wv[:, br], wv_d[:, br])
            nc.sync.dma_start(bv, bv_d)
            nc.sync.dma_start(muv, muv_d)

            # zero conv-halo borders + attbd (stt / block copies fill the
            # interior; only rows 0,65 and cols 0,65 must be zero).
            for j in range(4):
                nc.vector.memset(fp[j][:, 0:1, :], 0.0)
                nc.vector.memset(fp[j][:, 65:66, :], 0.0)
                nc.vector.memset(fp[j][:, :, 0:1], 0.0)
                nc.vector.memset(fp[j][:, :, 65:66], 0.0)
                nc.vector.memset(attbd[j], 0.0)

            # ================= Phase 1: transposed Q/K1/K2 + softmax pieces
            # One [128,768] PSUM tile spans 2 banks: Q(0:256)+K1(256:512) in
            # bank A, K2(512:768) in bank B. One start/stop per bank; other
            # first-writes rely on per-element has_written. One ACT does all
            # three exps.
            pv_ctx = tc.tile_pool(name="pv", bufs=3, space="PSUM")
            pv = pv_ctx.__enter__()
            with tc.tile_pool(name="pq", bufs=2, space="PSUM") as pq, \
                 tc.tile_pool(name="pss", bufs=1, space="PSUM") as pss:

                ps_s = pss.tile([128, 512], F32, tag="s")

                LAG = 2
                def emit_ssum(n):
                    # full-array ones lhsT: every output partition gets the
                    # same column sums; row 0 is read back. (1-col lhsT
                    # forces col_grp masks: ~+100ns on it and its successor.)
                    nc.tensor.matmul(ps_s, ones, qk[:, n, 256:768],
                                     start=(n == 0), stop=(n == NT - 1))

                gk = dict(skip_group_check=True)
                for n in range(NT):
                    pqk = pq.tile([128, 768], F32, tag="qk")
                    psq, psk1, psk2 = pqk[:, 0:256], pqk[:, 256:512], pqk[:, 512:768]
                    for ci in range(4):
                        lhsT = fbf[:, ci, n * 128:(n + 1) * 128]
                        nc.tensor.matmul(psq, lhsT, wq[:, ci, :],
                                         start=(ci == 0), stop=False, **gk)
                        if ci < 2:
                            nc.tensor.matmul(psk1, lhsT, wk[:, 0, ci, :],
                                             start=False, stop=False, **gk)
                        else:
                            nc.tensor.matmul(psk2, lhsT, wk[:, 1, ci - 2, :],
                                             start=(ci == 2), stop=(ci == 3), **gk)
                    # bias: full-array rank-128 matmul (ones.T @ (bq/128 rows))
                    nc.tensor.matmul(psq, ones, bqr, start=False, stop=True, **gk)

                    act = nc.scalar.activation(qk[:, n, :], pqk, AF.Exp)
                    if n == 8:
                        wc_anchor = act

                    # per-head softmax denominator + normalize (in place)
                    q3 = qk[:, n, 0:256].rearrange("p (h e) -> p h e", h=NH)
                    rq = sml.tile([128, NH], F32, tag="rq")
                    nc.vector.tensor_reduce(rq, q3, axis=AX.X, op=ALU.add)
                    rr = sml.tile([128, NH], F32, tag="rr")
                    nc.vector.reciprocal(rr, rq)
                    nc.vector.tensor_mul(q3, q3, rr.to_broadcast([128, NH, D]))

                    if n >= LAG:
                        emit_ssum(n - LAG)
                for n in range(NT - LAG, NT):
                    emit_ssum(n)

                # 1/S row -> per-partition columns (tiny SBUF->SBUF DMAs)
                scale = {}
                rs = sml.tile([1, 512], F32, tag="rs")
                nc.vector.reciprocal(rs, ps_s[0:1, :])
                for br in range(2):
                    for m in range(2):
                        col = sml.tile([128, 1], F32, tag="scat")
                        nc.sync.dma_start(
                            col, rs[0:1, br * 256 + m * 128:br * 256 + (m + 1) * 128])
                        sc = sml.tile([128, 1], F32, tag="scale")
                        nc.vector.tensor_mul(sc, col, muv)   # mu / S_d
                        scale[(br, m)] = sc

            # conv weights: start loading mid-phase-1 (nosync dep keeps them
            # out of the preamble DMA window), done long before the conv.
            from concourse.tile import add_dep_helper
            for ci in range(4):
                d = nc.sync.dma_start(wc[:, ci, :, :], wc_d[:, ci, :, :])
                add_dep_helper(d.ins, wc_anchor.ins, sync=False,
                               reason="defer wc load past preamble")
            d = nc.sync.dma_start(bns, bns_d)
            add_dep_helper(d.ins, wc_anchor.ins, sync=False, reason="defer")
            d = nc.sync.dma_start(bnb, bnb_d)
            add_dep_helper(d.ins, wc_anchor.ins, sync=False, reason="defer")

            # ================= Phase 2: V convs + AttT + block-diag
            def emit_v(pv, br, vsb, fci0):
                for m in range(2):
                    for n8 in range(8):
                        psv = pv.tile([128, 512], F32, tag="v",
                                      name=f"psv{br}{m}{n8}")
                        for ci in range(2):
                            nc.tensor.matmul(
                                psv, wv[:, br, ci, m * 128:(m + 1) * 128],
                                fbf[:, fci0 + ci, n8 * 512:(n8 + 1) * 512],
                                start=(ci == 0), stop=(ci == 1))
                        nc.scalar.activation(
                            vsb[:, m, n8 * 512:(n8 + 1) * 512], psv,
                            AF.Identity, bias=bv[:, br, m:m + 1])

            emit_v(pv, 0, vsb1, 0)

            with tc.tile_pool(name="pa", bufs=4, space="PSUM") as pa:
                psa = {}
                for g in range(2):
                    for br in range(2):
                        p = pa.tile([128, 128], F32, tag="a", name=f"psa{br}{g}")
                        psa[(br, g)] = p
                for g in range(2):
                    for n in range(NT):
                        lhsT = qk[:, n, g * 128:(g + 1) * 128]
                        for br in range(2):
                            nc.tensor.matmul(
                                psa[(br, g)], lhsT,
                                qk[:, n, 256 + br * 256 + g * 128:
                                   256 + br * 256 + (g + 1) * 128],
                                start=(n == 0), stop=(n == NT - 1))

                emit_v(pv, 1, vsb2, 2)

                # diag blocks into the pre-zeroed block-diag tiles
                for (br, g), p in psa.items():
                    t = attbd[2 * br + g]
                    for hb in range(4):
                        hs = slice(hb * 32, (hb + 1) * 32)
                        nc.any.tensor_copy(t[hs, hs], p[hs, hs])
            pv_ctx.__exit__(None, None, None)

            # ================= Phase 2b/3: Xw + fused epilogue, interleaved
            # with the conv3x3 output chunks. Xw chunks emit n8-major so all
            # four fp tiles grow top-down together; conv group k (output
            # rows 8k..8k+7) needs epilogue rows n8<=k+1 only. The epilogue
            # alternates Vector/GpSimd so the PE stream never waits on it.
            stt_cnt = 0

            def emit_xw_chunk(px, br, g, n8):
                nonlocal stt_cnt
                vsb, fci0 = ((vsb1, 0), (vsb2, 2))[br]
                pxt = px.tile([128, 512], F32, tag="x")
                nc.tensor.matmul(
                    pxt, attbd[2 * br + g],
                    vsb[:, g, n8 * 512:(n8 + 1) * 512],
                    start=True, stop=True)
                # fused = (Xw_raw * mu/S_d) + F  -> padded layout
                j = 2 * br + g
                out = fp[j][:, 1 + n8 * 8:9 + n8 * 8, 1:65]
                fres = fbf[:, fci0 + g, n8 * 512:(n8 + 1) * 512]
                if stt_cnt % 2 == 0:
                    nc.vector.scalar_tensor_tensor(
                        out=out, in0=pxt, scalar=scale[(br, g)], in1=fres,
                        op0=ALU.mult, op1=ALU.add)
                else:
                    # GPSIMD can't read PSUM: Scalar evacuates+scales, then
                    # GpSimd does the all-SBUF residual add.
                    xt = sml.tile([128, 512], BF16, tag="xt")
                    nc.scalar.activation(xt, pxt, AF.Identity,
                                         scale=scale[(br, g)])
                    nc.gpsimd.tensor_add(out, xt, fres)
                stt_cnt += 1

            def emit_conv_group(pc, kk, m, dma_i):
                pst = pc.tile([128, 512], F32, tag="c", name=f"psc{kk}{m}")
                first, last = (0, 0, 0), (3, 2, 2)
                for ci in range(4):
                    for dy in range(3):
                        for dx in range(3):
                            nc.tensor.matmul(
                                pst, wc[:, ci, (dy * 3 + dx) * 2 + m, :],
                                fp[ci][:, kk * 8 + dy:kk * 8 + dy + 8,
                                       dx:dx + 64],
                                start=((ci, dy, dx) == first),
                                stop=((ci, dy, dx) == last))
                ysb = sml.tile([128, 512], F32, tag="y")
                nc.scalar.activation(ysb, pst, AF.Relu,
                                     bias=bnb[:, m:m + 1],
                                     scale=bns[:, m:m + 1])
                eng = nc.sync if dma_i % 2 == 0 else nc.gpsimd
                eng.dma_start(
                    y_d[m * 128:(m + 1) * 128, kk * 512:(kk + 1) * 512], ysb)

            with tc.tile_pool(name="px", bufs=4, space="PSUM") as px, \
                 tc.tile_pool(name="pc", bufs=3, space="PSUM") as pc:
                conv_done = 0
                for n8 in range(8):
                    for br in range(2):
                        for g in range(2):
                            emit_xw_chunk(px, br, g, n8)
                    # conv group k reads fp rows from epilogue chunks
                    # n8 in {k-1, k, k+1}: emit it only after chunk k+1
                    while conv_done < n8:
                        kk = conv_done
                        emit_conv_group(pc, kk, 0, 2 * kk)
                        emit_conv_group(pc, kk, 1, 2 * kk + 1)
                        conv_done += 1
                while conv_done < 8:
                    kk = conv_done
                    emit_conv_group(pc, kk, 0, 2 * kk)
                    emit_conv_group(pc, kk, 1, 2 * kk + 1)
                    conv_done += 1
    nc.compile()
    return nc


_PROGRAM = None


def _get_program():
    global _PROGRAM
    if _PROGRAM is None:
        _PROGRAM = _build_program()
    return _PROGRAM


def kernel(F1, F2, Wq, bq, Wk1, bk1, Wv1, bv1, Wk2, bk2, Wv2, bv2,
           mu, Wc, gamma, beta, rmean, rvar):
    import os
    import sys
    if "antenv.axon_hooks" not in sys.modules:
        try:
            import antenv.axon_hooks  # noqa: F401
        except ImportError:
            # no profiling hook available: make sure a stray BASS_TRACE
            # can't route run_bass_kernel_spmd into the hook import
            os.environ["BASS_NEVER_TRACE"] = "1"
    f32 = np.float32
    F1 = np.asarray(F1, f32)
    F2 = np.asarray(F2, f32)

    def tile_T(w):   # [O, Cin] -> [128, Cin//128, O] (lhsT tiles)
        wt = np.ascontiguousarray(np.asarray(w, f32).T)      # [Cin, O]
        cin, o = wt.shape
        return wt.reshape(cin // 128, 128, o).transpose(1, 0, 2).astype(_bf)

    wq_h = np.ascontiguousarray(tile_T(Wq))                  # [128, 4, 256]
    wk_h = np.ascontiguousarray(
        np.stack([tile_T(Wk1), tile_T(Wk2)], axis=1))        # [128,2,2,256]
    wv_h = np.ascontiguousarray(
        np.stack([tile_T(Wv1), tile_T(Wv2)], axis=1))

    Wc = np.asarray(Wc, f32)                                 # [256, 512, 3, 3]
    # wc[p, ci, (dy*3+dx)*2+m, col] = Wc[m*128+col, ci*128+p, dy, dx]
    wc_h = Wc.reshape(2, 128, 4, 128, 3, 3)                  # m,col,ci,p,dy,dx
    wc_h = wc_h.transpose(3, 2, 4, 5, 0, 1)                  # p,ci,dy,dx,m,col
    wc_h = np.ascontiguousarray(
        wc_h.reshape(128, 4, 18, 128)).astype(_bf)

    bqr_h = np.ascontiguousarray(
        np.tile((np.asarray(bq, f32) / 128.0).reshape(1, 256),
                (128, 1))).astype(_bf)
    # bv_h[p, br, m] = bv_br[m*128 + p]
    bv_h = np.ascontiguousarray(
        np.stack([np.asarray(bv1, f32), np.asarray(bv2, f32)],
                 axis=0).reshape(2, 2, 128).transpose(2, 0, 1))
    inv = np.asarray(gamma, f32) / np.sqrt(np.asarray(rvar, f32) + BN_EPS)
    b2 = np.asarray(beta, f32) - np.asarray(rmean, f32) * inv
    bns_h = np.ascontiguousarray(inv.reshape(2, 128).T)      # [128, 2]
    bnb_h = np.ascontiguousarray(b2.reshape(2, 128).T)
    muv_h = np.full((128, 1), np.asarray(mu, f32).reshape(-1)[0], f32)

    shared = dict(wq=wq_h, wk=wk_h, wv=wv_h, wc=wc_h, bqr=bqr_h, bv=bv_h,
                  bns=bns_h, bnb=bnb_h, muv=muv_h)
    in_maps = [dict(f1=np.ascontiguousarray(F1[b].reshape(C, HW)).astype(_bf),
                    f2=np.ascontiguousarray(F2[b].reshape(C, HW)).astype(_bf),
                    **shared) for b in range(N_CORES)]

    nc = _get_program()
    res = run_bass_kernel_spmd(nc, in_maps, list(range(N_CORES)))
    kernel.last_results = res

    out = np.stack([res.results[b]["y"] for b in range(N_CORES)])
    return out.reshape(B, C, H, W)


kernel.last_results = None


# revision 14
# speedup vs baseline: 1.1347x; 1.0102x over previous
"""Trainium2 Bass kernel for the CGIM sparse-attention block.

Per-sample math (reference):
  Qf = Wq @ [F1;F2] + bq            (1x1 conv, transposed-layout on device)
  Qs = softmax_d(Qf per head)
  per branch i: K = Wk_i @ F_i (+bk_i cancels), V = Wv_i @ F_i + bv_i
                Ks = softmax_hw(K);  Att = Ks @ Qs;  Xw = Att @ V
  fused = concat(mu*X1 + F1, mu*X2 + F2)
  y = relu(BN(conv3x3(fused, Wc)))

Sharding: data-parallel over batch (B=8) across the 8 NeuronCores; weights
replicated. Each core computes one sample end to end; no collectives.

Key device-side algebra:
 - K softmax bias cancels (constant along softmax axis) -> dropped.
 - K softmax denominator S_d is postponed all the way to the Xw epilogue,
   where d is the partition axis (fused scale mu/S_d).
 - Q/K computed directly in transposed [hw, c] layout (stationary = F tile),
   so Q's per-head softmax is a free-dim segmented reduce and the Att
   contraction over hw needs no transposes at all.
 - AttT computed directly as lhsT for the Xw matmul; only the 32x32
   diagonal (per-head) blocks are copied into a zeroed block-diag tile.
 - conv3x3 = 36 accumulated shifted 1x1 matmuls over a zero-padded
   [66 x 72] image layout; BN+ReLU folded into the PSUM->SBUF ACT.
All matmul operands bf16, fp32 PSUM accumulation.

Scheduling notes (perf):
 - Q-bias and K-colsum matmuls use a full 128x128 ones matrix as lhsT:
   1-row/1-col stationaries force row_grp/col_grp masks and each mask
   switch costs ~+100ns on the next matmul.
 - Dummy warmup matmuls run during the DMA preamble so the PE HAM clock
   gate is already at 8/8 when phase 1 starts.
 - Xw chunks are emitted n8-major and the conv3x3 is emitted per-output-
   chunk (36-matmul accumulation groups) interleaved with them, so the
   PE never waits on the Vector/GpSimd epilogue and the conv tail
   (ACT+DMA) pipelines under the next group's matmuls.
 - The fused-residual epilogue alternates Vector / GpSimd so neither
   paces the PE.
"""

import numpy as np
import ml_dtypes

import concourse.bass as bass
import concourse.mybir as mybir
import concourse.tile as tile
from concourse import bacc
from concourse.bass_utils import run_bass_kernel_spmd

BF16 = mybir.dt.bfloat16
F32 = mybir.dt.float32
AF = mybir.ActivationFunctionType
ALU = mybir.AluOpType
AX = mybir.AxisListType

B, C, H, W = 8, 256, 64, 64
HW = H * W                  # 4096
NH, D = 8, 32               # heads, per-head dim
NT = HW // 128              # 32 hw-tiles of 128
PH, PW = H + 2, 72          # padded conv image (66 rows x 72 cols)
N_CORES = 8
BN_EPS = 1e-5

_bf = ml_dtypes.bfloat16


def _build_program() -> bass.Bass:
    nc = bacc.Bacc("TRN2", target_bir_lowering=False)

    # ---- DRAM I/O (per core) ----
    f1_d = nc.dram_tensor("f1", [C, HW], BF16, kind="ExternalInput").ap()
    f2_d = nc.dram_tensor("f2", [C, HW], BF16, kind="ExternalInput").ap()
    wq_d = nc.dram_tensor("wq", [128, 4, 256], BF16, kind="ExternalInput").ap()
    wk_d = nc.dram_tensor("wk", [128, 2, 2, 256], BF16, kind="ExternalInput").ap()
    wv_d = nc.dram_tensor("wv", [128, 2, 2, 256], BF16, kind="ExternalInput").ap()
    wc_d = nc.dram_tensor("wc", [128, 4, 18, 128], BF16, kind="ExternalInput").ap()
    bqr_d = nc.dram_tensor("bqr", [128, 256], BF16, kind="ExternalInput").ap()
    bv_d = nc.dram_tensor("bv", [128, 2, 2], F32, kind="ExternalInput").ap()
    bns_d = nc.dram_tensor("bns", [128, 2], F32, kind="ExternalInput").ap()
    bnb_d = nc.dram_tensor("bnb", [128, 2], F32, kind="ExternalInput").ap()
    muv_d = nc.dram_tensor("muv", [128, 1], F32, kind="ExternalInput").ap()
    y_d = nc.dram_tensor("y", [C, HW], F32, kind="ExternalOutput").ap()

    with tile.TileContext(nc) as tc:
        with tc.tile_pool(name="per", bufs=1) as per, \
             tc.tile_pool(name="sml", bufs=4) as sml:

            # ---- persistent SBUF tiles ----
            wq = per.tile([128, 4, 256], BF16)
            wk = per.tile([128, 2, 2, 256], BF16)
            wv = per.tile([128, 2, 2, 256], BF16)
            wc = per.tile([128, 4, 18, 128], BF16)
            bqr = per.tile([128, 256], BF16)
            bv = per.tile([128, 2, 2], F32)
            bns = per.tile([128, 2], F32)
            bnb = per.tile([128, 2], F32)
            muv = per.tile([128, 1], F32)

            ones = per.tile([128, 128], BF16)
            nc.vector.memset(ones, 1.0)

            fbf = per.tile([128, 4, HW], BF16)        # [F1;F2] as 4 ci-tiles
            # qk[:, n, 0:256]=exp(QfT) (normalized in place),
            # [:, n, 256:512]=exp(K1fT), [:, n, 512:768]=exp(K2fT)
            qk = per.tile([128, NT, 768], BF16)
            vsb1 = per.tile([128, 2, HW], BF16)       # V1, 2 m-groups
            vsb2 = per.tile([128, 2, HW], BF16)
            fp = [per.tile([128, PH, PW], BF16, tag=f"fp{j}", name=f"fp{j}")
                  for j in range(4)]
            attbd = [per.tile([128, 128], BF16, tag=f"abd{j}", name=f"abd{j}")
                     for j in range(4)]

            # PE warmup: dummy matmuls during the DMA preamble keep the PE
            # busy through the HAM window so phase 1 starts at 2.4 GHz.
            warm_ctx = tc.tile_pool(name="warm", bufs=1, space="PSUM")
            warm = warm_ctx.__enter__()
            wt = warm.tile([128, 64], F32)
            for _ in range(25):
                nc.tensor.matmul(wt, ones, ones[:, 0:64],
                                 start=True, stop=True)
            warm_ctx.__exit__(None, None, None)

            # ---- preamble DMA: first-needed tiles fan out across queues ----
            # DMA *issue* costs ~0.65us per dma_start per sequencer, so the
            # first phase-1 deps go one-per-queue in consumption order, then
            # bulk loads follow.
            f_src = [f1_d, f1_d, f2_d, f2_d]
            nc.sync.dma_start(fbf[:, 0, 0:256], f1_d[0:128, 0:256])
            nc.gpsimd.dma_start(fbf[:, 1, 0:256], f1_d[128:256, 0:256])
            nc.scalar.dma_start(wq[:, 0], wq_d[:, 0])
            nc.scalar.dma_start(fbf[:, 2, 0:256], f2_d[0:128, 0:256])
            nc.sync.dma_start(wk[:, 0], wk_d[:, 0])
            nc.gpsimd.dma_start(fbf[:, 3, 0:256], f2_d[128:256, 0:256])
            nc.scalar.dma_start(wq[:, 1:4], wq_d[:, 1:4])
            nc.gpsimd.dma_start(wk[:, 1], wk_d[:, 1])
            nc.scalar.dma_start(bqr, bqr_d)

            bounds = [256, 1280, 2304, 3328, 4096]
            k = 0
            for ch in range(len(bounds) - 1):
                lo, hi = bounds[ch], bounds[ch + 1]
                for ci in range(4):
                    half = (ci % 2) * 128
                    eng = nc.sync if k % 2 == 0 else nc.gpsimd
                    eng.dma_start(fbf[:, ci, lo:hi],
                                  f_src[ci][half:half + 128, lo:hi])
                    k += 1

            for br in range(2):
                eng = nc.sync if br == 0 else nc.gpsimd
                eng.dma_start(wv[:, br], wv_d[:, br])
            nc.sync.dma_start(bv, bv_d)
            nc.sync.dma_start(muv, muv_d)

            # zero conv-halo borders + attbd (stt / block copies fill the
            # interior; only rows 0,65 and cols 0,65 must be zero).
            for j in range(4):
                nc.vector.memset(fp[j][:, 0:1, :], 0.0)
                nc.vector.memset(fp[j][:, 65:66, :], 0.0)
                nc.vector.memset(fp[j][:, :, 0:1], 0.0)
                nc.vector.memset(fp[j][:, :, 65:66], 0.0)
                nc.vector.memset(attbd[j], 0.0)

            # ================= Phase 1: transposed Q/K1/K2 + softmax pieces
            # One [128,768] PSUM tile spans 2 banks: Q(0:256)+K1(256:512) in
            # bank A, K2(512:768) in bank B. One start/stop per bank; other
            # first-writes rely on per-element has_written. One ACT does all
            # three exps.
            pv_ctx = tc.tile_pool(name="pv", bufs=3, space="PSUM")
            pv = pv_ctx.__enter__()
            with tc.tile_pool(name="pq", bufs=2, space="PSUM") as pq:

                gk = dict(skip_group_check=True)
                for n in range(NT):
                    pqk = pq.tile([128, 768], F32, tag="qk")
                    psq, psk1, psk2 = pqk[:, 0:256], pqk[:, 256:512], pqk[:, 512:768]
                    for ci in range(4):
                        lhsT = fbf[:, ci, n * 128:(n + 1) * 128]
                        nc.tensor.matmul(psq, lhsT, wq[:, ci, :],
                                         start=(ci == 0), stop=False, **gk)
                        if ci < 2:
                            nc.tensor.matmul(psk1, lhsT, wk[:, 0, ci, :],
                                             start=False, stop=False, **gk)
                        else:
                            nc.tensor.matmul(psk2, lhsT, wk[:, 1, ci - 2, :],
                                             start=(ci == 2), stop=(ci == 3), **gk)
                    # bias: full-array rank-128 matmul (ones.T @ (bq/128 rows))
                    nc.tensor.matmul(psq, ones, bqr, start=False, stop=True, **gk)

                    act = nc.scalar.activation(qk[:, n, :], pqk, AF.Exp)
                    if n == 8:
                        wc_anchor = act

                    # per-head softmax denominator + normalize (in place)
                    q3 = qk[:, n, 0:256].rearrange("p (h e) -> p h e", h=NH)
                    rq = sml.tile([128, NH], F32, tag="rq")
                    nc.vector.tensor_reduce(rq, q3, axis=AX.X, op=ALU.add)
                    rr = sml.tile([128, NH], F32, tag="rr")
                    nc.vector.reciprocal(rr, rq)
                    # alternate engines so Vector doesn't pace the phase
                    meng = nc.vector if n % 2 == 0 else nc.gpsimd
                    meng.tensor_mul(q3, q3, rr.to_broadcast([128, NH, D]))

            # conv weights: start loading mid-phase-1 (nosync dep keeps them
            # out of the preamble DMA window), done long before the conv.
            from concourse.tile import add_dep_helper
            for ci in range(4):
                d = nc.sync.dma_start(wc[:, ci, :, :], wc_d[:, ci, :, :])
                add_dep_helper(d.ins, wc_anchor.ins, sync=False,
                               reason="defer wc load past preamble")
            d = nc.sync.dma_start(bns, bns_d)
            add_dep_helper(d.ins, wc_anchor.ins, sync=False, reason="defer")
            d = nc.sync.dma_start(bnb, bnb_d)
            add_dep_helper(d.ins, wc_anchor.ins, sync=False, reason="defer")

            # ================= Phase 2: V convs + AttT + block-diag
            def emit_v(pv, br, vsb, fci0):
                # evacuation alternates Scalar/Vector: one engine alone
                # (~700ns per [128,512] chunk) would pace the 432ns matmuls
                for m in range(2):
                    for n8 in range(8):
                        psv = pv.tile([128, 512], F32, tag="v",
                                      name=f"psv{br}{m}{n8}")
                        for ci in range(2):
                            nc.tensor.matmul(
                                psv, wv[:, br, ci, m * 128:(m + 1) * 128],
                                fbf[:, fci0 + ci, n8 * 512:(n8 + 1) * 512],
                                start=(ci == 0), stop=(ci == 1))
                        dst = vsb[:, m, n8 * 512:(n8 + 1) * 512]
                        if n8 % 2 == 0:
                            nc.scalar.activation(dst, psv, AF.Identity,
                                                 bias=bv[:, br, m:m + 1])
                        else:
                            nc.vector.tensor_scalar_add(dst, psv,
                                                        bv[:, br, m:m + 1])

            emit_v(pv, 0, vsb1, 0)

            with tc.tile_pool(name="pa", bufs=4, space="PSUM") as pa:
                psa = {}
                for g in range(2):
                    for br in range(2):
                        p = pa.tile([128, 128], F32, tag="a", name=f"psa{br}{g}")
                        psa[(br, g)] = p
                for g in range(2):
                    for n in range(NT):
                        lhsT = qk[:, n, g * 128:(g + 1) * 128]
                        for br in range(2):
                            nc.tensor.matmul(
                                psa[(br, g)], lhsT,
                                qk[:, n, 256 + br * 256 + g * 128:
                                   256 + br * 256 + (g + 1) * 128],
                                start=(n == 0), stop=(n == NT - 1))

                emit_v(pv, 1, vsb2, 2)

                # diag blocks into the pre-zeroed block-diag tiles
                for (br, g), p in psa.items():
                    t = attbd[2 * br + g]
                    for hb in range(4):
                        hs = slice(hb * 32, (hb + 1) * 32)
                        nc.any.tensor_copy(t[hs, hs], p[hs, hs])

                # K-softmax denominators for free: Qs head-rows sum to 1,
                # so the column sums of each diagonal Att^T block are
                # exactly S_d = sum_hw exp(K)[hw, d]. One N=1 matmul per
                # (br,g) replaces 32 [128,512] column-sum matmuls.
                scale = {}
                for (br, g) in psa:
                    # recycle the psa ring slots (same tag) for the S columns
                    pS = pa.tile([128, 128], F32, tag="a", name=f"ss{br}{g}")
                    pS = pS[:, 0:1]
                    nc.tensor.matmul(pS, attbd[2 * br + g], ones[:, 0:1],
                                     start=True, stop=True)
                    col = sml.tile([128, 1], F32, tag="scat")
                    nc.vector.reciprocal(col, pS)
                    sc = sml.tile([128, 1], F32, tag="scale")
                    nc.vector.tensor_mul(sc, col, muv)   # mu / S_d
                    scale[(br, g)] = sc
            pv_ctx.__exit__(None, None, None)

            # ================= Phase 2b/3: Xw + fused epilogue, interleaved
            # with the conv3x3 output chunks. Xw chunks emit n8-major so all
            # four fp tiles grow top-down together; conv group k (output
            # rows 8k..8k+7) needs epilogue rows n8<=k+1 only. The epilogue
            # alternates Vector/GpSimd so the PE stream never waits on it.
            stt_cnt = 0

            def emit_xw_chunk(px, br, g, n8):
                nonlocal stt_cnt
                vsb, fci0 = ((vsb1, 0), (vsb2, 2))[br]
                pxt = px.tile([128, 512], F32, tag="x")
                nc.tensor.matmul(
                    pxt, attbd[2 * br + g],
                    vsb[:, g, n8 * 512:(n8 + 1) * 512],
                    start=True, stop=True)
                # fused = (Xw_raw * mu/S_d) + F  -> padded layout
                j = 2 * br + g
                out = fp[j][:, 1 + n8 * 8:9 + n8 * 8, 1:65]
                fres = fbf[:, fci0 + g, n8 * 512:(n8 + 1) * 512]
                if stt_cnt % 2 == 0:
                    nc.vector.scalar_tensor_tensor(
                        out=out, in0=pxt, scalar=scale[(br, g)], in1=fres,
                        op0=ALU.mult, op1=ALU.add)
                else:
                    # GPSIMD can't read PSUM: Scalar evacuates+scales, then
                    # GpSimd does the all-SBUF residual add.
                    xt = sml.tile([128, 512], BF16, tag="xt")
                    nc.scalar.activation(xt, pxt, AF.Identity,
                                         scale=scale[(br, g)])
                    nc.gpsimd.tensor_add(out, xt, fres)
                stt_cnt += 1

            def emit_conv_group(pc, kk, m, dma_i):
                pst = pc.tile([128, 512], F32, tag="c", name=f"psc{kk}{m}")
                first, last = (0, 0, 0), (3, 2, 2)
                for ci in range(4):
                    for dy in range(3):
                        for dx in range(3):
                            nc.tensor.matmul(
                                pst, wc[:, ci, (dy * 3 + dx) * 2 + m, :],
                                fp[ci][:, kk * 8 + dy:kk * 8 + dy + 8,
                                       dx:dx + 64],
                                start=((ci, dy, dx) == first),
                                stop=((ci, dy, dx) == last))
                ysb = sml.tile([128, 512], F32, tag="y")
                nc.scalar.activation(ysb, pst, AF.Relu,
                                     bias=bnb[:, m:m + 1],
                                     scale=bns[:, m:m + 1])
                eng = nc.sync if dma_i % 2 == 0 else nc.gpsimd
                eng.dma_start(
                    y_d[m * 128:(m + 1) * 128, kk * 512:(kk + 1) * 512], ysb)

            with tc.tile_pool(name="px", bufs=4, space="PSUM") as px, \
                 tc.tile_pool(name="pc", bufs=3, space="PSUM") as pc:
                conv_done = 0
                for n8 in range(8):
                    for br in range(2):
                        for g in range(2):
                            emit_xw_chunk(px, br, g, n8)
                    # conv group k reads fp rows from epilogue chunks
                    # n8 in {k-1, k, k+1}: emit it only after chunk k+1
                    while conv_done < n8:
                        kk = conv_done
                        emit_conv_group(pc, kk, 0, 2 * kk)
                        emit_conv_group(pc, kk, 1, 2 * kk + 1)
                        conv_done += 1
                while conv_done < 8:
                    kk = conv_done
                    emit_conv_group(pc, kk, 0, 2 * kk)
                    emit_conv_group(pc, kk, 1, 2 * kk + 1)
                    conv_done += 1
    nc.compile()
    return nc


_PROGRAM = None


def _get_program():
    global _PROGRAM
    if _PROGRAM is None:
        _PROGRAM = _build_program()
    return _PROGRAM


def kernel(F1, F2, Wq, bq, Wk1, bk1, Wv1, bv1, Wk2, bk2, Wv2, bv2,
           mu, Wc, gamma, beta, rmean, rvar):
    import os
    import sys
    if "antenv.axon_hooks" not in sys.modules:
        try:
            import antenv.axon_hooks  # noqa: F401
        except ImportError:
            # no profiling hook available: make sure a stray BASS_TRACE
            # can't route run_bass_kernel_spmd into the hook import
            os.environ["BASS_NEVER_TRACE"] = "1"
    f32 = np.float32
    F1 = np.asarray(F1, f32)
    F2 = np.asarray(F2, f32)

    def tile_T(w):   # [O, Cin] -> [128, Cin//128, O] (lhsT tiles)
        wt = np.ascontiguousarray(np.asarray(w, f32).T)      # [Cin, O]
        cin, o = wt.shape
        return wt.reshape(cin // 128, 128, o).transpose(1, 0, 2).astype(_bf)

    wq_h = np.ascontiguousarray(tile_T(Wq))                  # [128, 4, 256]
    wk_h = np.ascontiguousarray(
        np.stack([tile_T(Wk1), tile_T(Wk2)], axis=1))        # [128,2,2,256]
    wv_h = np.ascontiguousarray(
        np.stack([tile_T(Wv1), tile_T(Wv2)], axis=1))

    Wc = np.asarray(Wc, f32)                                 # [256, 512, 3, 3]
    # wc[p, ci, (dy*3+dx)*2+m, col] = Wc[m*128+col, ci*128+p, dy, dx]
    wc_h = Wc.reshape(2, 128, 4, 128, 3, 3)                  # m,col,ci,p,dy,dx
    wc_h = wc_h.transpose(3, 2, 4, 5, 0, 1)                  # p,ci,dy,dx,m,col
    wc_h = np.ascontiguousarray(
        wc_h.reshape(128, 4, 18, 128)).astype(_bf)

    bqr_h = np.ascontiguousarray(
        np.tile((np.asarray(bq, f32) / 128.0).reshape(1, 256),
                (128, 1))).astype(_bf)
    # bv_h[p, br, m] = bv_br[m*128 + p]
    bv_h = np.ascontiguousarray(
        np.stack([np.asarray(bv1, f32), np.asarray(bv2, f32)],
                 axis=0).reshape(2, 2, 128).transpose(2, 0, 1))
    inv = np.asarray(gamma, f32) / np.sqrt(np.asarray(rvar, f32) + BN_EPS)
    b2 = np.asarray(beta, f32) - np.asarray(rmean, f32) * inv
    bns_h = np.ascontiguousarray(inv.reshape(2, 128).T)      # [128, 2]
    bnb_h = np.ascontiguousarray(b2.reshape(2, 128).T)
    muv_h = np.full((128, 1), np.asarray(mu, f32).reshape(-1)[0], f32)

    shared = dict(wq=wq_h, wk=wk_h, wv=wv_h, wc=wc_h, bqr=bqr_h, bv=bv_h,
                  bns=bns_h, bnb=bnb_h, muv=muv_h)
    in_maps = [dict(f1=np.ascontiguousarray(F1[b].reshape(C, HW)).astype(_bf),
                    f2=np.ascontiguousarray(F2[b].reshape(C, HW)).astype(_bf),
                    **shared) for b in range(N_CORES)]

    nc = _get_program()
    res = run_bass_kernel_spmd(nc, in_maps, list(range(N_CORES)))
    kernel.last_results = res

    out = np.stack([res.results[b]["y"] for b in range(N_CORES)])
    return out.reshape(B, C, H, W)


kernel.last_results = None
